# revision 5
# baseline (speedup 1.0000x reference)
"""Trainium2 Bass/Tile kernel: two chained VALID 3x3 convolutions.

    x  [N,3,256,256] --conv(w1)--> h [N,64,254,254] --conv(w2)--> out [N,128,252,252]

Data-parallel over 8 NeuronCores: batch N=16 -> 2 images per core, conv
weights replicated.

Perf structure (v3):

conv2 runs as 5 full-128-row matmul passes per 2-row output chunk (vs 6
mixed 128/64-row passes in v2), using two doubled SBUF buffers:

  H  [128p]: 0:64 = h,          64:128 = h shifted down 1 row
  Hb [128p]: 0:64 = h down 2,   64:128 = h down 2, left 1 col

  wp[dj] @ H(t, dj)   -> taps (0,dj)+(1,dj)   (3 passes)
  wb     @ Hb(t, 0)   -> taps (2,0)+(2,1)     (1 pass)
  ws     @ Hb(t, 1)   -> tap  (2,2)           (1 pass, top-half weights zero)

All five passes are K=128 at tile (128,128) so the PE never pays the
64<->128 tile-config switch.  Hb's column shift is baked in at copy time
by writing a flat per-partition byte stream at a +1-element offset.

conv1 runs as 4 concurrent row-tiled K=27 matmuls (PE 32-row quadrants
(0,0)/(32,0)/(64,0)/(96,0)), quadrupling conv1 throughput.  The im2col
buffer B1 is a flat per-partition byte stream holding full 512 B x rows;
the (di,dj) tap shift is a per-partition byte offset, so each of the
27x4 partition loads is ONE contiguous ~19.5 KB DMA packet from HBM.

Evictions / copies per conv1 chunk (PSUM P1 holds h twice via
column-duplicated conv1 weights, keeping everything partition-aligned):
  DVE:    P1[0:64]   -> H[0:64]            (rows 2j)
  ACT:    P1[64:128] -> H[64:128]          (rows 2j-1, the row shift)
  GPSIMD: H[0:64]    -> Hb[0:64]   flat    (rows -2, strip pieces)
  DMA:    H[0:64]    -> Hb[64:128] flat-1  (rows -2 cols +1, strip pieces)

conv2 PSUM is evicted f32->bf16 into a 6-row SBUF stage by DVE and DMA'd
to HBM as bf16 (host upcasts), halving output DMA traffic.  Lookahead is
one strip: phase s = conv2(s) + conv1(s+1) bursts + im2col(s+2) loads.
"""

from contextlib import ExitStack

import ml_dtypes
import numpy as np

import concourse.bass as bass
import concourse.mybir as mybir
import concourse.tile as tile
import concourse.bass_utils as bass_utils
from concourse import bacc

N_CORES = 8
FULL_N = 16
C0, C1, C2 = 3, 64, 128

H0 = W0 = 256
H1 = W1 = 254
H2 = W2 = 252
TY = 36                                  # conv2 output rows per strip
NPC = FULL_N // N_CORES                  # images per core
SPI = H2 // TY                           # strips per image (7)
NSTRIPS = NPC * SPI                      # strips per core (14)
NC1 = (TY + 2) // 2                      # conv1 2-row chunks per strip (19)
NC2 = TY // 2                            # conv2 2-row chunks per strip (18)
BLK = 3                                  # conv2 chunks per block
NB = NC2 // BLK                          # conv2 blocks per strip (6)
NQ = (NC1 + 3) // 4                      # conv1 quad-groups per strip (5)

B1_ROWS = 38                             # x rows loaded per im2col partition
B1_FLAT = 39 * W0                        # flat B1 elems/partition (spill pad)
HB_FLAT = TY * W1 + 2                    # flat Hb elems/partition

MODE = "bf16"


def _mm_dt():
    return mybir.dt.bfloat16 if MODE == "bf16" else mybir.dt.float32r


def _np_dt():
    return ml_dtypes.bfloat16 if MODE == "bf16" else np.float32


def _emit(ctx: ExitStack, tc: tile.TileContext, out, x, w1q, wp, wb, ws, mm_dt):
    nc = tc.nc
    f32 = mybir.dt.float32

    wpool = ctx.enter_context(tc.tile_pool(name="weights", bufs=1))
    b1pool = ctx.enter_context(tc.tile_pool(name="b1", bufs=3))
    hpool = ctx.enter_context(tc.tile_pool(name="h", bufs=2))
    hbpool = ctx.enter_context(tc.tile_pool(name="hb", bufs=2))
    opool = ctx.enter_context(tc.tile_pool(name="o2", bufs=4))
    ps1 = ctx.enter_context(tc.tile_pool(name="ps1", bufs=4, space="PSUM"))
    ps2 = ctx.enter_context(tc.tile_pool(name="ps2", bufs=4, space="PSUM"))

    # conv1 weights: [27, 128] (output cols duplicated), one copy per PE
    # row-quadrant so 4 chunk matmuls run concurrently via row tiling.
    w1sb = wpool.tile([128, 128], mm_dt, tag="w1")
    for q in range(4):
        nc.sync.dma_start(w1sb[32 * q:32 * q + 27, :], w1q)
    wp_sb = []
    for dj in range(3):
        wpt = wpool.tile([128, C2], mm_dt, tag=f"wp{dj}")
        nc.sync.dma_start(wpt[:], wp[dj])
        wp_sb.append(wpt)
    wb_sb = wpool.tile([128, C2], mm_dt, tag="wb")
    nc.sync.dma_start(wb_sb[:], wb)
    ws_sb = wpool.tile([128, C2], mm_dt, tag="ws")
    nc.sync.dma_start(ws_sb[:], ws)

    def strip_of(s):
        n, k = divmod(s, SPI)
        return n, k * TY

    B1_tiles, H_tiles, Hb_tiles = {}, {}, {}

    def emit_im2col(s):
        """Flat im2col: partition 32q + 3*(3di+dj) + c holds x rows
        y0+di..y0+di+38 as full 512B rows at element offset (2-dj); the
        conv1 moving AP then reads tap (di,dj) at uniform offset col 2."""
        n, y0 = strip_of(s)
        B1 = b1pool.tile([128, B1_FLAT], mm_dt, tag="b1", name=f"B1_{s}")
        B1_tiles[s] = B1
        for q in range(4):
            for t9 in range(9):
                di, dj = divmod(t9, 3)
                p = 32 * q + 3 * t9
                off = 2 - dj
                nc.sync.dma_start(
                    B1[p:p + 3, off:off + B1_ROWS * W0],
                    x[n, :, y0 + di:y0 + di + B1_ROWS, :])

    def emit_conv1_quad(s, g):
        """conv1 chunks 4g..4g+3 as concurrent row-tiled K=27 matmuls."""
        if g == 0:
            H_tiles[s] = hpool.tile([128, B1_ROWS, W1], mm_dt, tag="h",
                                    name=f"H{s}")
            Hb_tiles[s] = hbpool.tile([128, HB_FLAT], mm_dt, tag="hb",
                                      name=f"Hb{s}")
        H = H_tiles[s]
        B1v = B1_tiles[s].rearrange("p (r c) -> p r c", c=W0)
        P1s = []
        for i in range(4):
            j = 4 * g + i
            if j >= NC1:
                break
            P1 = ps1.tile([128, 2, W1], f32, tag="p1", name=f"P1_{i}")
            nc.tensor.matmul(
                P1[:], w1sb[32 * i:32 * i + 27, :],
                B1v[32 * i:32 * i + 27, 2 * j:2 * j + 2, 2:W0],
                start=True, stop=True, tile_position=(32 * i, 0))
            P1s.append((j, P1))
        for j, P1 in P1s:
            r = 2 * j
            nc.vector.tensor_copy(H[0:C1, r:r + 2, :], P1[0:C1])
            if j == 0:
                nc.scalar.copy(H[C1:128, 0:1, :], P1[C1:128, 1:2, :])
            else:
                nc.scalar.copy(H[C1:128, r - 1:r + 1, :], P1[C1:128])

    # piece p may only cover H rows produced by conv1 quads 0..p
    # (quad p ends at chunk 4p+3 = h row 8p+8): Hb rows < 8p+6.
    HB_PIECES = [(0, 6), (6, 14), (14, 22), (22, 30), (30, TY)]

    def emit_hb_piece(s, p):
        """Hb rows [a,b): lo half = h rows +2 (GPSIMD), hi half = h rows
        +2 cols +1 via the flat -1-element offset (DMA, cross-partition)."""
        a, b = HB_PIECES[p]
        H, Hb = H_tiles[s], Hb_tiles[s]
        src = H[0:C1, 2 + a:2 + b, :]
        nc.gpsimd.tensor_copy(
            Hb[0:C1, 1 + W1 * a:1 + W1 * b],
            src.rearrange("p r c -> p (r c)"))
        nc.scalar.dma_start(Hb[C1:128, W1 * a:W1 * b], src)

    def emit_conv2_block(s, k):
        n, y0 = strip_of(s)
        H = H_tiles[s]
        Hbv = Hb_tiles[s][:, 1:1 + TY * W1].rearrange("p (r c) -> p r c", c=W1)
        OS = opool.tile([C2, 2 * BLK, W2], mm_dt, tag="os")
        for c in range(BLK):
            t = 2 * (BLK * k + c)
            P2 = ps2.tile([C2, 2, W2], f32, tag="p2", name=f"P2_{c}")
            for dj in range(3):
                nc.tensor.matmul(P2[:], wp_sb[dj][:],
                                 H[:, t:t + 2, dj:dj + W2],
                                 start=(dj == 0), stop=False,
                                 skip_group_check=True)
            nc.tensor.matmul(P2[:], wb_sb[:], Hbv[:, t:t + 2, 0:W2],
                             start=False, stop=False, skip_group_check=True)
            nc.tensor.matmul(P2[:], ws_sb[:], Hbv[:, t:t + 2, 1:1 + W2],
                             start=False, stop=True, skip_group_check=True)
            nc.vector.tensor_copy(OS[:, 2 * c:2 * c + 2, :], P2[:])
        y = y0 + 2 * BLK * k
        nc.scalar.dma_start(out[n, :, y:y + 2 * BLK, :], OS[:])

    # prologue: strips 0/1 im2col, all of conv1(0).
    emit_im2col(0)
    emit_im2col(1)
    for g in range(NQ):
        emit_conv1_quad(0, g)
        emit_hb_piece(0, g)

    for s in range(NSTRIPS):
        if s + 2 < NSTRIPS:
            emit_im2col(s + 2)
        for k in range(NB):
            emit_conv2_block(s, k)
            if k < NQ and s + 1 < NSTRIPS:
                emit_conv1_quad(s + 1, k)
                emit_hb_piece(s + 1, k)


def build(mm_dt=None):
    if mm_dt is None:
        mm_dt = _mm_dt()
    nc = bacc.Bacc("TRN2", target_bir_lowering=False, debug=False,
                   num_devices=N_CORES)
    x = nc.dram_tensor("x", [NPC, C0, H0, W0], mm_dt,
                       kind="ExternalInput").ap()
    w1q = nc.dram_tensor("w1q", [27, 128], mm_dt, kind="ExternalInput").ap()
    wp = nc.dram_tensor("wp", [3, 128, C2], mm_dt, kind="ExternalInput").ap()
    wb = nc.dram_tensor("wb", [128, C2], mm_dt, kind="ExternalInput").ap()
    ws = nc.dram_tensor("ws", [128, C2], mm_dt, kind="ExternalInput").ap()
    out = nc.dram_tensor("out", [NPC, C2, H2, W2], mm_dt,
                         kind="ExternalOutput").ap()
    with tile.TileContext(nc) as tc:
        with ExitStack() as ctx:
            _emit(ctx, tc, out, x, w1q, wp, wb, ws, mm_dt)
    nc.compile()
    return nc


def host_round(a: np.ndarray) -> np.ndarray:
    """Cast fp32 to the matmul storage dtype (bf16 cast, or tf32 rounding)."""
    a = np.ascontiguousarray(a, dtype=np.float32)
    if MODE == "bf16":
        return a.astype(ml_dtypes.bfloat16)
    b = a.view(np.uint32).copy()
    b += 0xFFF + ((b >> 13) & 1)
    b &= np.uint32(0xFFFFE000)
    return b.view(np.float32)


def pack_weights(w1: np.ndarray, w2: np.ndarray):
    """Host-side repack so every device DMA is contiguous.

    w1q[p, o] = w1[o%64, c, di, dj], p = (di*3+dj)*3 + c  (cols duplicated)
    wp[dj, k, o]: k<64 -> w2[o, k, 0, dj]; k>=64 -> w2[o, k-64, 1, dj]
    wb[k, o]:     k<64 -> w2[o, k, 2, 0];  k>=64 -> w2[o, k-64, 2, 1]
    ws[k, o]:     k<64 -> 0;               k>=64 -> w2[o, k-64, 2, 2]
    """
    w1 = np.ascontiguousarray(np.asarray(w1), dtype=np.float32)
    w2 = np.ascontiguousarray(np.asarray(w2), dtype=np.float32)
    w1t = w1.transpose(2, 3, 1, 0).reshape(27, C1)
    w1q = np.concatenate([w1t, w1t], axis=1)
    wp = np.empty((3, 128, C2), np.float32)
    wp[:, :C1] = w2[:, :, 0, :].transpose(2, 1, 0)
    wp[:, C1:] = w2[:, :, 1, :].transpose(2, 1, 0)
    wb = np.empty((128, C2), np.float32)
    wb[:C1] = w2[:, :, 2, 0].T
    wb[C1:] = w2[:, :, 2, 1].T
    ws = np.zeros((128, C2), np.float32)
    ws[C1:] = w2[:, :, 2, 2].T
    return (host_round(w1q), host_round(wp), host_round(wb), host_round(ws))


_NC_CACHE: dict = {}


def _get_nc():
    key = ("main", MODE, TY)
    if key not in _NC_CACHE:
        _NC_CACHE[key] = build()
    return _NC_CACHE[key]


def run(x, w1, w2, trace: bool = False):
    """Shard, run on 8 cores, gather.  Returns (out, BassKernelResults)."""
    x = np.ascontiguousarray(np.asarray(x), dtype=np.float32)
    assert x.shape == (FULL_N, C0, H0, W0), x.shape
    w1q, wp, wb, ws = pack_weights(w1, w2)
    xs = host_round(x).reshape(N_CORES, NPC, C0, H0, W0)
    in_maps = [
        {"x": np.ascontiguousarray(xs[c]), "w1q": w1q, "wp": wp,
         "wb": wb, "ws": ws}
        for c in range(N_CORES)
    ]
    nc = _get_nc()
    res = bass_utils.run_bass_kernel_spmd(
        nc, in_maps, core_ids=list(range(N_CORES)), trace=trace)
    out = np.concatenate(
        [np.asarray(r["out"], dtype=np.float32) for r in res.results], axis=0)
    return out, res


def kernel(x, w1, w2):
    out, _ = run(x, w1, w2, trace=False)
    return out


# revision 7
# speedup vs baseline: 1.1619x; 1.1619x over previous
"""Trainium2 Bass/Tile kernel: two chained VALID 3x3 convolutions.

    x  [N,3,256,256] --conv(w1)--> h [N,64,254,254] --conv(w2)--> out [N,128,252,252]

Data-parallel over 8 NeuronCores: batch N=16 -> 2 images per core, conv
weights replicated.

Perf structure (v3):

conv2 runs as 5 full-128-row matmul passes per 2-row output chunk (vs 6
mixed 128/64-row passes in v2), using two doubled SBUF buffers:

  H  [128p]: 0:64 = h,          64:128 = h shifted down 1 row
  Hb [128p]: 0:64 = h down 2,   64:128 = h down 2, left 1 col

  wp[dj] @ H(t, dj)   -> taps (0,dj)+(1,dj)   (3 passes)
  wb     @ Hb(t, 0)   -> taps (2,0)+(2,1)     (1 pass)
  ws     @ Hb(t, 1)   -> tap  (2,2)           (1 pass, top-half weights zero)

All five passes are K=128 at tile (128,128) so the PE never pays the
64<->128 tile-config switch.  Hb's column shift is baked in at copy time
by writing a flat per-partition byte stream at a +1-element offset.

conv1 runs as 4 concurrent row-tiled K=27 matmuls (PE 32-row quadrants
(0,0)/(32,0)/(64,0)/(96,0)), quadrupling conv1 throughput.  The im2col
buffer B1 is a flat per-partition byte stream holding full 512 B x rows;
the (di,dj) tap shift is a per-partition byte offset, so each of the
27x4 partition loads is ONE contiguous ~19.5 KB DMA packet from HBM.

Evictions / copies per conv1 chunk (PSUM P1 holds h twice via
column-duplicated conv1 weights, keeping everything partition-aligned):
  DVE:    P1[0:64]   -> H[0:64]            (rows 2j)
  ACT:    P1[64:128] -> H[64:128]          (rows 2j-1, the row shift)
  GPSIMD: H[0:64]    -> Hb[0:64]   flat    (rows -2, strip pieces)
  DMA:    H[0:64]    -> Hb[64:128] flat-1  (rows -2 cols +1, strip pieces)

conv2 PSUM is evicted f32->bf16 into a 6-row SBUF stage by DVE and DMA'd
to HBM as bf16 (host upcasts), halving output DMA traffic.  Lookahead is
one strip: phase s = conv2(s) + conv1(s+1) bursts + im2col(s+2) loads.
"""

from contextlib import ExitStack

import ml_dtypes
import numpy as np

import concourse.bass as bass
import concourse.mybir as mybir
import concourse.tile as tile
import concourse.bass_utils as bass_utils
from concourse import bacc

N_CORES = 8
FULL_N = 16
C0, C1, C2 = 3, 64, 128

H0 = W0 = 256
H1 = W1 = 254
H2 = W2 = 252
TY = 36                                  # conv2 output rows per strip
NPC = FULL_N // N_CORES                  # images per core
SPI = H2 // TY                           # strips per image (7)
NSTRIPS = NPC * SPI                      # strips per core (14)
NC1 = (TY + 2) // 2                      # conv1 2-row chunks per strip (19)
NC2 = TY // 2                            # conv2 2-row chunks per strip (18)
BLK = 3                                  # conv2 chunks per block
NB = NC2 // BLK                          # conv2 blocks per strip (6)
NQ = (NC1 + 3) // 4                      # conv1 quad-groups per strip (5)

B1_ROWS = 38                             # x rows loaded per im2col partition
B1_FLAT = 39 * W0                        # flat B1 elems/partition (spill pad)
HB_FLAT = TY * W1 + 2                    # flat Hb elems/partition

MODE = "bf16"


def _mm_dt():
    return mybir.dt.bfloat16 if MODE == "bf16" else mybir.dt.float32r


def _np_dt():
    return ml_dtypes.bfloat16 if MODE == "bf16" else np.float32


def _emit(ctx: ExitStack, tc: tile.TileContext, out, x, w1q, wp, wb, ws, mm_dt):
    nc = tc.nc
    f32 = mybir.dt.float32

    wpool = ctx.enter_context(tc.tile_pool(name="weights", bufs=1))
    b1pool = ctx.enter_context(tc.tile_pool(name="b1", bufs=3))
    hpool = ctx.enter_context(tc.tile_pool(name="h", bufs=2))
    hbpool = ctx.enter_context(tc.tile_pool(name="hb", bufs=2))
    opool = ctx.enter_context(tc.tile_pool(name="o2", bufs=4))
    ps1 = ctx.enter_context(tc.tile_pool(name="ps1", bufs=4, space="PSUM"))
    ps2 = ctx.enter_context(tc.tile_pool(name="ps2", bufs=4, space="PSUM"))

    # conv1 weights: [27, 128] (output cols duplicated), one copy per PE
    # row-quadrant so 4 chunk matmuls run concurrently via row tiling.
    w1sb = wpool.tile([128, 128], mm_dt, tag="w1")
    for q in range(4):
        nc.sync.dma_start(w1sb[32 * q:32 * q + 27, :], w1q)
    wp_sb = []
    for dj in range(3):
        wpt = wpool.tile([128, C2], mm_dt, tag=f"wp{dj}")
        nc.sync.dma_start(wpt[:], wp[dj])
        wp_sb.append(wpt)
    wb_sb = wpool.tile([128, C2], mm_dt, tag="wb")
    nc.sync.dma_start(wb_sb[:], wb)
    ws_sb = wpool.tile([128, C2], mm_dt, tag="ws")
    nc.sync.dma_start(ws_sb[:], ws)

    def strip_of(s):
        n, k = divmod(s, SPI)
        return n, k * TY

    B1_tiles, H_tiles, Hb_tiles = {}, {}, {}

    def emit_im2col(s):
        """Flat im2col: partition 32q + 3*(3di+dj) + c holds x rows
        y0+di..y0+di+38 as full 512B rows at element offset (2-dj); the
        conv1 moving AP then reads tap (di,dj) at uniform offset col 2."""
        n, y0 = strip_of(s)
        B1 = b1pool.tile([128, B1_FLAT], mm_dt, tag="b1", name=f"B1_{s}")
        B1_tiles[s] = B1
        for q in range(4):
            for t9 in range(9):
                di, dj = divmod(t9, 3)
                p = 32 * q + 3 * t9
                off = 2 - dj
                nc.sync.dma_start(
                    B1[p:p + 3, off:off + B1_ROWS * W0],
                    x[n, :, y0 + di:y0 + di + B1_ROWS, :])

    def emit_conv1_quad(s, g):
        """conv1 chunks 4g..4g+3 as concurrent row-tiled K=27 matmuls."""
        if g == 0:
            H_tiles[s] = hpool.tile([128, B1_ROWS, W1], mm_dt, tag="h",
                                    name=f"H{s}")
            Hb_tiles[s] = hbpool.tile([128, HB_FLAT], mm_dt, tag="hb",
                                      name=f"Hb{s}")
        H = H_tiles[s]
        B1v = B1_tiles[s].rearrange("p (r c) -> p r c", c=W0)
        P1s = []
        for i in range(4):
            j = 4 * g + i
            if j >= NC1:
                break
            P1 = ps1.tile([128, 2, W1], f32, tag="p1", name=f"P1_{i}")
            nc.tensor.matmul(
                P1[:], w1sb[32 * i:32 * i + 27, :],
                B1v[32 * i:32 * i + 27, 2 * j:2 * j + 2, 2:W0],
                start=True, stop=True, tile_position=(32 * i, 0))
            P1s.append((j, P1))
        for j, P1 in P1s:
            r = 2 * j
            nc.vector.tensor_copy(H[0:C1, r:r + 2, :], P1[0:C1])
            if j == 0:
                nc.scalar.copy(H[C1:128, 0:1, :], P1[C1:128, 1:2, :])
            else:
                nc.scalar.copy(H[C1:128, r - 1:r + 1, :], P1[C1:128])

    # piece p may only cover H rows produced by conv1 quads 0..p
    # (quad p ends at chunk 4p+3 = h row 8p+8): Hb rows < 8p+6.
    HB_PIECES = [(0, 6), (6, 14), (14, 22), (22, 30), (30, TY)]

    def emit_hb_piece(s, p):
        """Hb rows [a,b): lo half = h rows +2, hi half = h rows +2 cols
        +1 via the flat -1-element offset.  Both are contiguous
        SBUF->SBUF DMAs sourced from H's lower half."""
        a, b = HB_PIECES[p]
        H, Hb = H_tiles[s], Hb_tiles[s]
        src = H[0:C1, 2 + a:2 + b, :]
        nc.sync.dma_start(Hb[0:C1, 1 + W1 * a:1 + W1 * b], src)
        nc.scalar.dma_start(Hb[C1:128, W1 * a:W1 * b], src)

    def emit_conv2_block(s, k):
        n, y0 = strip_of(s)
        H = H_tiles[s]
        Hbv = Hb_tiles[s][:, 1:1 + TY * W1].rearrange("p (r c) -> p r c", c=W1)
        OS = opool.tile([C2, 2 * BLK, W2], mm_dt, tag="os")
        for c in range(BLK):
            t = 2 * (BLK * k + c)
            P2 = ps2.tile([C2, 2, W2], f32, tag="p2", name=f"P2_{c}")
            for dj in range(3):
                nc.tensor.matmul(P2[:], wp_sb[dj][:],
                                 H[:, t:t + 2, dj:dj + W2],
                                 start=(dj == 0), stop=False,
                                 skip_group_check=True)
            nc.tensor.matmul(P2[:], wb_sb[:], Hbv[:, t:t + 2, 0:W2],
                             start=False, stop=False, skip_group_check=True)
            nc.tensor.matmul(P2[:], ws_sb[:], Hbv[:, t:t + 2, 1:1 + W2],
                             start=False, stop=True, skip_group_check=True)
            nc.vector.tensor_copy(OS[0:C1, 2 * c:2 * c + 2, :], P2[0:C1])
            nc.scalar.copy(OS[C1:C2, 2 * c:2 * c + 2, :], P2[C1:C2])
        y = y0 + 2 * BLK * k
        nc.scalar.dma_start(out[n, :, y:y + 2 * BLK, :], OS[:])

    # prologue: strips 0/1 im2col, all of conv1(0).
    emit_im2col(0)
    emit_im2col(1)
    for g in range(NQ):
        emit_conv1_quad(0, g)
        emit_hb_piece(0, g)

    for s in range(NSTRIPS):
        if s + 2 < NSTRIPS:
            emit_im2col(s + 2)
        for k in range(NB):
            emit_conv2_block(s, k)
            if k < NQ and s + 1 < NSTRIPS:
                emit_conv1_quad(s + 1, k)
                emit_hb_piece(s + 1, k)


def build(mm_dt=None):
    if mm_dt is None:
        mm_dt = _mm_dt()
    nc = bacc.Bacc("TRN2", target_bir_lowering=False, debug=False,
                   num_devices=N_CORES)
    x = nc.dram_tensor("x", [NPC, C0, H0, W0], mm_dt,
                       kind="ExternalInput").ap()
    w1q = nc.dram_tensor("w1q", [27, 128], mm_dt, kind="ExternalInput").ap()
    wp = nc.dram_tensor("wp", [3, 128, C2], mm_dt, kind="ExternalInput").ap()
    wb = nc.dram_tensor("wb", [128, C2], mm_dt, kind="ExternalInput").ap()
    ws = nc.dram_tensor("ws", [128, C2], mm_dt, kind="ExternalInput").ap()
    out = nc.dram_tensor("out", [NPC, C2, H2, W2], mm_dt,
                         kind="ExternalOutput").ap()
    with tile.TileContext(nc) as tc:
        with ExitStack() as ctx:
            _emit(ctx, tc, out, x, w1q, wp, wb, ws, mm_dt)
    nc.compile()
    return nc


def host_round(a: np.ndarray) -> np.ndarray:
    """Cast fp32 to the matmul storage dtype (bf16 cast, or tf32 rounding)."""
    a = np.ascontiguousarray(a, dtype=np.float32)
    if MODE == "bf16":
        return a.astype(ml_dtypes.bfloat16)
    b = a.view(np.uint32).copy()
    b += 0xFFF + ((b >> 13) & 1)
    b &= np.uint32(0xFFFFE000)
    return b.view(np.float32)


def pack_weights(w1: np.ndarray, w2: np.ndarray):
    """Host-side repack so every device DMA is contiguous.

    w1q[p, o] = w1[o%64, c, di, dj], p = (di*3+dj)*3 + c  (cols duplicated)
    wp[dj, k, o]: k<64 -> w2[o, k, 0, dj]; k>=64 -> w2[o, k-64, 1, dj]
    wb[k, o]:     k<64 -> w2[o, k, 2, 0];  k>=64 -> w2[o, k-64, 2, 1]
    ws[k, o]:     k<64 -> 0;               k>=64 -> w2[o, k-64, 2, 2]
    """
    w1 = np.ascontiguousarray(np.asarray(w1), dtype=np.float32)
    w2 = np.ascontiguousarray(np.asarray(w2), dtype=np.float32)
    w1t = w1.transpose(2, 3, 1, 0).reshape(27, C1)
    w1q = np.concatenate([w1t, w1t], axis=1)
    wp = np.empty((3, 128, C2), np.float32)
    wp[:, :C1] = w2[:, :, 0, :].transpose(2, 1, 0)
    wp[:, C1:] = w2[:, :, 1, :].transpose(2, 1, 0)
    wb = np.empty((128, C2), np.float32)
    wb[:C1] = w2[:, :, 2, 0].T
    wb[C1:] = w2[:, :, 2, 1].T
    ws = np.zeros((128, C2), np.float32)
    ws[C1:] = w2[:, :, 2, 2].T
    return (host_round(w1q), host_round(wp), host_round(wb), host_round(ws))


_NC_CACHE: dict = {}


def _get_nc():
    key = ("main", MODE, TY)
    if key not in _NC_CACHE:
        _NC_CACHE[key] = build()
    return _NC_CACHE[key]


def run(x, w1, w2, trace: bool = False):
    """Shard, run on 8 cores, gather.  Returns (out, BassKernelResults)."""
    x = np.ascontiguousarray(np.asarray(x), dtype=np.float32)
    assert x.shape == (FULL_N, C0, H0, W0), x.shape
    w1q, wp, wb, ws = pack_weights(w1, w2)
    xs = host_round(x).reshape(N_CORES, NPC, C0, H0, W0)
    in_maps = [
        {"x": np.ascontiguousarray(xs[c]), "w1q": w1q, "wp": wp,
         "wb": wb, "ws": ws}
        for c in range(N_CORES)
    ]
    nc = _get_nc()
    res = bass_utils.run_bass_kernel_spmd(
        nc, in_maps, core_ids=list(range(N_CORES)), trace=trace)
    out = np.concatenate(
        [np.asarray(r["out"], dtype=np.float32) for r in res.results], axis=0)
    return out, res


def kernel(x, w1, w2):
    out, _ = run(x, w1, w2, trace=False)
    return out


# revision 13
# speedup vs baseline: 1.4125x; 1.2157x over previous
"""Trainium2 Bass/Tile kernel: two chained VALID 3x3 convolutions.

    x  [N,3,256,256] --conv(w1)--> h [N,64,254,254] --conv(w2)--> out [N,128,252,252]

Data-parallel over 8 NeuronCores: batch N=16 -> 2 images per core, conv
weights replicated.

Perf structure (v3):

conv2 runs as 5 full-128-row matmul passes per 2-row output chunk (vs 6
mixed 128/64-row passes in v2), using two doubled SBUF buffers:

  H  [128p]: 0:64 = h,          64:128 = h shifted down 1 row
  Hb [128p]: 0:64 = h down 2,   64:128 = h down 2, left 1 col

  wp[dj] @ H(t, dj)   -> taps (0,dj)+(1,dj)   (3 passes)
  wb     @ Hb(t, 0)   -> taps (2,0)+(2,1)     (1 pass)
  ws     @ Hb(t, 1)   -> tap  (2,2)           (1 pass, top-half weights zero)

All five passes are K=128 at tile (128,128) so the PE never pays the
64<->128 tile-config switch.  Hb's column shift is baked in at copy time
by writing a flat per-partition byte stream at a +1-element offset.

conv1 runs as 4 concurrent row-tiled K=27 matmuls (PE 32-row quadrants
(0,0)/(32,0)/(64,0)/(96,0)), quadrupling conv1 throughput.  The im2col
buffer B1 is a flat per-partition byte stream holding full 512 B x rows;
the (di,dj) tap shift is a per-partition byte offset, so each of the
27x4 partition loads is ONE contiguous ~19.5 KB DMA packet from HBM.

Evictions / copies per conv1 chunk (PSUM P1 holds h twice via
column-duplicated conv1 weights, keeping everything partition-aligned):
  DVE:    P1[0:64]   -> H[0:64]            (rows 2j)
  ACT:    P1[64:128] -> H[64:128]          (rows 2j-1, the row shift)
  GPSIMD: H[0:64]    -> Hb[0:64]   flat    (rows -2, strip pieces)
  DMA:    H[0:64]    -> Hb[64:128] flat-1  (rows -2 cols +1, strip pieces)

conv2 PSUM is evicted f32->bf16 into a 6-row SBUF stage by DVE and DMA'd
to HBM as bf16 (host upcasts), halving output DMA traffic.  Lookahead is
one strip: phase s = conv2(s) + conv1(s+1) bursts + im2col(s+2) loads.
"""

from contextlib import ExitStack

import ml_dtypes
import numpy as np

import concourse.bass as bass
import concourse.mybir as mybir
import concourse.tile as tile
import concourse.bass_utils as bass_utils
from concourse import bacc

N_CORES = 8
FULL_N = 16
C0, C1, C2 = 3, 64, 128

H0 = W0 = 256
H1 = W1 = 254
H2 = W2 = 252
TY = 36                                  # conv2 output rows per strip
NPC = FULL_N // N_CORES                  # images per core
SPI = H2 // TY                           # strips per image (7)
NSTRIPS = NPC * SPI                      # strips per core (14)
NC1 = (TY + 2) // 2                      # conv1 2-row chunks per strip (19)
NC2 = TY // 2                            # conv2 2-row chunks per strip (18)
BLK = 3                                  # conv2 chunks per block
NB = NC2 // BLK                          # conv2 blocks per strip (6)
NG = (NC1 + 1) // 2                      # conv1 pair-groups per strip (10)

B1_ROWS = 38                             # x rows loaded per im2col partition
B1_FLAT = 39 * W0                        # flat B1 elems/partition (spill pad)
HB_FLAT = TY * W1 + 2                    # flat Hb elems/partition

MODE = "bf16"


def _mm_dt():
    return mybir.dt.bfloat16 if MODE == "bf16" else mybir.dt.float32r


def _np_dt():
    return ml_dtypes.bfloat16 if MODE == "bf16" else np.float32


def _emit(ctx: ExitStack, tc: tile.TileContext, out, x, w1q, wp, wb, ws, mm_dt):
    nc = tc.nc
    f32 = mybir.dt.float32

    wpool = ctx.enter_context(tc.tile_pool(name="weights", bufs=1))
    b1pool = ctx.enter_context(tc.tile_pool(name="b1", bufs=3))
    hpool = ctx.enter_context(tc.tile_pool(name="h", bufs=2))
    hbpool = ctx.enter_context(tc.tile_pool(name="hb", bufs=2))
    opool = ctx.enter_context(tc.tile_pool(name="o2", bufs=4))
    ps1 = ctx.enter_context(tc.tile_pool(name="ps1", bufs=4, space="PSUM"))
    ps2 = ctx.enter_context(tc.tile_pool(name="ps2", bufs=4, space="PSUM"))

    # conv1 weights: [27, 128] (output cols duplicated), one copy per PE
    # 32-row group so 2 chunk matmuls run concurrently via row tiling.
    w1sb = wpool.tile([64, 128], mm_dt, tag="w1")
    for q in range(2):
        nc.sync.dma_start(w1sb[32 * q:32 * q + 27, :], w1q)
    wp_sb = []
    for dj in range(3):
        wpt = wpool.tile([128, C2], mm_dt, tag=f"wp{dj}")
        nc.sync.dma_start(wpt[:], wp[dj])
        wp_sb.append(wpt)
    wb_sb = wpool.tile([128, C2], mm_dt, tag="wb")
    nc.sync.dma_start(wb_sb[:], wb)
    ws_sb = wpool.tile([128, C2], mm_dt, tag="ws")
    nc.sync.dma_start(ws_sb[:], ws)

    def strip_of(s):
        n, k = divmod(s, SPI)
        return n, k * TY

    B1_tiles, H_tiles, Hb_tiles = {}, {}, {}

    def emit_im2col(s):
        """Flat im2col: partition 32q + 3*(3di+dj) + c holds x rows
        y0+di..y0+di+38 as full 512B rows at element offset (2-dj); the
        conv1 moving AP then reads tap (di,dj) at uniform offset col 2."""
        n, y0 = strip_of(s)
        B1 = b1pool.tile([64, B1_FLAT], mm_dt, tag="b1", name=f"B1_{s}")
        B1_tiles[s] = B1
        for q in range(2):
            for t9 in range(9):
                di, dj = divmod(t9, 3)
                p = 32 * q + 3 * t9
                off = 2 - dj
                eng = nc.sync if (t9 % 2 == 0) else nc.scalar
                eng.dma_start(
                    B1[p:p + 3, off:off + B1_ROWS * W0],
                    x[n, :, y0 + di:y0 + di + B1_ROWS, :])

    def emit_conv1_pair(s, g):
        """conv1 chunks 2g..2g+1 as concurrent row-tiled K=27 matmuls.

        Each chunk's PSUM bank is read by exactly one engine (H0+H1+Hb
        lo evictions chained) -- two engines on one bank serialize on
        the bank read port."""
        if g == 0:
            H_tiles[s] = hpool.tile([128, B1_ROWS, W1], mm_dt, tag="h",
                                    name=f"H{s}")
            Hb_tiles[s] = hbpool.tile([128, HB_FLAT], mm_dt, tag="hb",
                                      name=f"Hb{s}")
        H = H_tiles[s]
        Hb = Hb_tiles[s]
        B1v = B1_tiles[s].rearrange("p (r c) -> p r c", c=W0)
        P1s = []
        for i in range(2):
            j = 2 * g + i
            if j >= NC1:
                break
            P1 = ps1.tile([128, 2, W1], f32, tag="p1", name=f"P1_{i}")
            nc.tensor.matmul(
                P1[:], w1sb[32 * i:32 * i + 27, :],
                B1v[32 * i:32 * i + 27, 2 * j:2 * j + 2, 2:W0],
                start=True, stop=True, tile_position=(32 * i, 0))
            P1s.append((j, P1))
        for j, P1 in P1s:
            r = 2 * j
            eng = nc.vector if j % 2 == 0 else nc.scalar
            copy = eng.tensor_copy if j % 2 == 0 else eng.copy
            copy(H[0:C1, r:r + 2, :], P1[0:C1])
            if j == 0:
                copy(H[C1:128, 0:1, :], P1[C1:128, 1:2, :])
            else:
                copy(H[C1:128, r - 1:r + 1, :], P1[C1:128])
            if j >= 1:
                copy(Hb[0:C1, 1 + W1 * (r - 2):1 + W1 * r],
                     P1[0:C1].rearrange("p r c -> p (r c)"))

    # piece p may only cover H rows produced by conv1 pair-groups
    # 0..2p+1 (chunk 4p+3 = h row 8p+8): Hb rows < 8p+6.
    HB_PIECES = [(0, 6), (6, 14), (14, 22), (22, 30), (30, TY)]

    def emit_hb_piece(s, p):
        """Hb hi half rows [a,b) = h rows +2 cols +1 via the flat
        -1-element offset: contiguous SBUF->SBUF DMA from H's lower
        half, split into two chains so two DMA engines work on it."""
        a, b = HB_PIECES[p]
        H, Hb = H_tiles[s], Hb_tiles[s]
        for h0, h1, eng in ((0, 32, nc.sync), (32, C1, nc.scalar)):
            eng.dma_start(Hb[C1 + h0:C1 + h1, W1 * a:W1 * b],
                          H[h0:h1, 2 + a:2 + b, :])

    def emit_conv2_block(s, k):
        n, y0 = strip_of(s)
        H = H_tiles[s]
        Hbv = Hb_tiles[s][:, 1:1 + TY * W1].rearrange("p (r c) -> p r c", c=W1)
        OS = opool.tile([C2, 2 * BLK, W2], mm_dt, tag="os")
        for c in range(BLK):
            t = 2 * (BLK * k + c)
            P2 = ps2.tile([C2, 2, W2], f32, tag="p2", name=f"P2_{c}")
            for dj in range(3):
                nc.tensor.matmul(P2[:], wp_sb[dj][:],
                                 H[:, t:t + 2, dj:dj + W2],
                                 start=(dj == 0), stop=False,
                                 skip_group_check=True)
            nc.tensor.matmul(P2[:], wb_sb[:], Hbv[:, t:t + 2, 0:W2],
                             start=False, stop=False, skip_group_check=True)
            nc.tensor.matmul(P2[:], ws_sb[:], Hbv[:, t:t + 2, 1:1 + W2],
                             start=False, stop=True, skip_group_check=True)
            if (BLK * k + c) % 2 == 0:
                nc.vector.tensor_copy(OS[:, 2 * c:2 * c + 2, :], P2[:])
            else:
                nc.scalar.copy(OS[:, 2 * c:2 * c + 2, :], P2[:])
        y = y0 + 2 * BLK * k
        # 4 chains so 4 DMA engines carry the output in parallel
        for h0, h1, eng in ((0, 32, nc.sync), (32, 64, nc.scalar),
                            (64, 96, nc.sync), (96, 128, nc.scalar)):
            eng.dma_start(out[n, h0:h1, y:y + 2 * BLK, :], OS[h0:h1])

    # prologue: strips 0/1 im2col, all of conv1(0).
    emit_im2col(0)
    emit_im2col(1)
    for g in range(NG):
        emit_conv1_pair(0, g)
        if g % 2 == 1:
            emit_hb_piece(0, g // 2)

    for s in range(NSTRIPS):
        if s + 2 < NSTRIPS:
            emit_im2col(s + 2)
        for k in range(NB):
            emit_conv2_block(s, k)
            if k < 5 and s + 1 < NSTRIPS:
                emit_conv1_pair(s + 1, 2 * k)
                emit_conv1_pair(s + 1, 2 * k + 1)
                emit_hb_piece(s + 1, k)


def build(mm_dt=None):
    if mm_dt is None:
        mm_dt = _mm_dt()
    nc = bacc.Bacc("TRN2", target_bir_lowering=False, debug=False,
                   num_devices=N_CORES)
    x = nc.dram_tensor("x", [NPC, C0, H0, W0], mm_dt,
                       kind="ExternalInput").ap()
    w1q = nc.dram_tensor("w1q", [27, 128], mm_dt, kind="ExternalInput").ap()
    wp = nc.dram_tensor("wp", [3, 128, C2], mm_dt, kind="ExternalInput").ap()
    wb = nc.dram_tensor("wb", [128, C2], mm_dt, kind="ExternalInput").ap()
    ws = nc.dram_tensor("ws", [128, C2], mm_dt, kind="ExternalInput").ap()
    out = nc.dram_tensor("out", [NPC, C2, H2, W2], mm_dt,
                         kind="ExternalOutput").ap()
    with tile.TileContext(nc) as tc:
        with ExitStack() as ctx:
            _emit(ctx, tc, out, x, w1q, wp, wb, ws, mm_dt)
    nc.compile()
    return nc


def host_round(a: np.ndarray) -> np.ndarray:
    """Cast fp32 to the matmul storage dtype (bf16 cast, or tf32 rounding)."""
    a = np.ascontiguousarray(a, dtype=np.float32)
    if MODE == "bf16":
        return a.astype(ml_dtypes.bfloat16)
    b = a.view(np.uint32).copy()
    b += 0xFFF + ((b >> 13) & 1)
    b &= np.uint32(0xFFFFE000)
    return b.view(np.float32)


def pack_weights(w1: np.ndarray, w2: np.ndarray):
    """Host-side repack so every device DMA is contiguous.

    w1q[p, o] = w1[o%64, c, di, dj], p = (di*3+dj)*3 + c  (cols duplicated)
    wp[dj, k, o]: k<64 -> w2[o, k, 0, dj]; k>=64 -> w2[o, k-64, 1, dj]
    wb[k, o]:     k<64 -> w2[o, k, 2, 0];  k>=64 -> w2[o, k-64, 2, 1]
    ws[k, o]:     k<64 -> 0;               k>=64 -> w2[o, k-64, 2, 2]
    """
    w1 = np.ascontiguousarray(np.asarray(w1), dtype=np.float32)
    w2 = np.ascontiguousarray(np.asarray(w2), dtype=np.float32)
    w1t = w1.transpose(2, 3, 1, 0).reshape(27, C1)
    w1q = np.concatenate([w1t, w1t], axis=1)
    wp = np.empty((3, 128, C2), np.float32)
    wp[:, :C1] = w2[:, :, 0, :].transpose(2, 1, 0)
    wp[:, C1:] = w2[:, :, 1, :].transpose(2, 1, 0)
    wb = np.empty((128, C2), np.float32)
    wb[:C1] = w2[:, :, 2, 0].T
    wb[C1:] = w2[:, :, 2, 1].T
    ws = np.zeros((128, C2), np.float32)
    ws[C1:] = w2[:, :, 2, 2].T
    return (host_round(w1q), host_round(wp), host_round(wb), host_round(ws))


_NC_CACHE: dict = {}


def _get_nc():
    key = ("main", MODE, TY)
    if key not in _NC_CACHE:
        _NC_CACHE[key] = build()
    return _NC_CACHE[key]


def run(x, w1, w2, trace: bool = False):
    """Shard, run on 8 cores, gather.  Returns (out, BassKernelResults)."""
    x = np.ascontiguousarray(np.asarray(x), dtype=np.float32)
    assert x.shape == (FULL_N, C0, H0, W0), x.shape
    w1q, wp, wb, ws = pack_weights(w1, w2)
    xs = host_round(x).reshape(N_CORES, NPC, C0, H0, W0)
    in_maps = [
        {"x": np.ascontiguousarray(xs[c]), "w1q": w1q, "wp": wp,
         "wb": wb, "ws": ws}
        for c in range(N_CORES)
    ]
    nc = _get_nc()
    res = bass_utils.run_bass_kernel_spmd(
        nc, in_maps, core_ids=list(range(N_CORES)), trace=trace)
    out = np.concatenate(
        [np.asarray(r["out"], dtype=np.float32) for r in res.results], axis=0)
    return out, res


def kernel(x, w1, w2):
    out, _ = run(x, w1, w2, trace=False)
    return out


# revision 14
# speedup vs baseline: 1.5493x; 1.0968x over previous
"""Trainium2 Bass/Tile kernel: two chained VALID 3x3 convolutions.

    x  [N,3,256,256] --conv(w1)--> h [N,64,254,254] --conv(w2)--> out [N,128,252,252]

Data-parallel over 8 NeuronCores: batch N=16 -> 2 images per core, conv
weights replicated.

Perf structure (v5):

conv2 runs as 6 K=128 matmul passes per 2-row output chunk against a
doubled SBUF buffer H (partitions 0:64 = h, 64:128 = h shifted down one
row):

  wp[dj] @ H(t,   dj) -> taps (0,dj)+(1,dj)   (3 pair passes)
  ws[dj] @ H(t+1, dj) -> tap  (2,dj)          (3 passes, top-half weights
                                               zero so K stays 128)

Everything is tile-config (128,128), so the PE never pays the 64<->128
row-config switch the v2 kernel paid per block.

conv1 runs as 2 concurrent row-tiled K=27 matmuls (PE row groups (0,0)
and (32,0)).  The im2col buffer B1 is a flat per-partition byte stream
holding full 512 B x rows; the (di,dj) tap shift is a per-partition
byte offset, so each 3-partition load is ONE contiguous ~19.5 KB DMA
packet from HBM (38 rows of x at full width).

Copies (conv1 PSUM P1 holds h twice via column-duplicated conv1
weights, keeping everything partition-aligned):
  DVE/ACT (alternating per chunk): P1[0:64] -> H[0:64]   (the h rows)
  DMA (strip pieces):  H[0:64] rows+1 -> H[64:128]       (the row shift)
  DVE/ACT (alternating per chunk): conv2 PSUM -> OS bf16

conv2 output is staged in SBUF as bf16 and DMA'd to HBM in 6-row
pieces split into 4 per-channel-group chains (parallel DMA engines);
the host upcasts to f32.  Lookahead is one strip: phase s = conv2(s) +
conv1(s+1) bursts + im2col(s+2) loads.
"""

from contextlib import ExitStack

import ml_dtypes
import numpy as np

import concourse.bass as bass
import concourse.mybir as mybir
import concourse.tile as tile
import concourse.bass_utils as bass_utils
from concourse import bacc

N_CORES = 8
FULL_N = 16
C0, C1, C2 = 3, 64, 128

H0 = W0 = 256
H1 = W1 = 254
H2 = W2 = 252
TY = 36                                  # conv2 output rows per strip
NPC = FULL_N // N_CORES                  # images per core
SPI = H2 // TY                           # strips per image (7)
NSTRIPS = NPC * SPI                      # strips per core (14)
NC1 = (TY + 2) // 2                      # conv1 2-row chunks per strip (19)
NC2 = TY // 2                            # conv2 2-row chunks per strip (18)
BLK = 3                                  # conv2 chunks per block
NB = NC2 // BLK                          # conv2 blocks per strip (6)
NG = (NC1 + 1) // 2                      # conv1 pair-groups per strip (10)

B1_ROWS = 38                             # x rows loaded per im2col partition
B1_FLAT = 39 * W0                        # flat B1 elems/partition (spill pad)

MODE = "bf16"


def _mm_dt():
    return mybir.dt.bfloat16 if MODE == "bf16" else mybir.dt.float32r


def _emit(ctx: ExitStack, tc: tile.TileContext, out, x, w1q, wp, ws, mm_dt):
    nc = tc.nc
    f32 = mybir.dt.float32

    wpool = ctx.enter_context(tc.tile_pool(name="weights", bufs=1))
    b1pool = ctx.enter_context(tc.tile_pool(name="b1", bufs=3))
    hpool = ctx.enter_context(tc.tile_pool(name="h", bufs=2))
    opool = ctx.enter_context(tc.tile_pool(name="o2", bufs=4))
    ps1 = ctx.enter_context(tc.tile_pool(name="ps1", bufs=4, space="PSUM"))
    ps2 = ctx.enter_context(tc.tile_pool(name="ps2", bufs=4, space="PSUM"))

    # conv1 weights: [27, 128] (output cols duplicated), one copy per PE
    # 32-row group so 2 chunk matmuls run concurrently via row tiling.
    w1sb = wpool.tile([64, 128], mm_dt, tag="w1")
    for q in range(2):
        nc.sync.dma_start(w1sb[32 * q:32 * q + 27, :], w1q)
    wp_sb, ws_sb = [], []
    for dj in range(3):
        wpt = wpool.tile([128, C2], mm_dt, tag=f"wp{dj}")
        nc.sync.dma_start(wpt[:], wp[dj])
        wp_sb.append(wpt)
        wst = wpool.tile([128, C2], mm_dt, tag=f"ws{dj}")
        nc.sync.dma_start(wst[:], ws[dj])
        ws_sb.append(wst)

    def strip_of(s):
        n, k = divmod(s, SPI)
        return n, k * TY

    B1_tiles, H_tiles = {}, {}

    def emit_im2col(s):
        """Flat im2col: partition 32q + 3*(3di+dj) + c holds x rows
        y0+di..y0+di+38 as full 512B rows at element offset (2-dj); the
        conv1 moving AP then reads tap (di,dj) at uniform offset col 2."""
        n, y0 = strip_of(s)
        B1 = b1pool.tile([64, B1_FLAT], mm_dt, tag="b1", name=f"B1_{s}")
        B1_tiles[s] = B1
        for q in range(2):
            for t9 in range(9):
                di, dj = divmod(t9, 3)
                p = 32 * q + 3 * t9
                off = 2 - dj
                eng = nc.sync if (t9 % 2 == 0) else nc.scalar
                eng.dma_start(
                    B1[p:p + 3, off:off + B1_ROWS * W0],
                    x[n, :, y0 + di:y0 + di + B1_ROWS, :])

    def emit_conv1_pair(s, g):
        """conv1 chunks 2g..2g+1 as concurrent row-tiled K=27 matmuls."""
        if g == 0:
            H_tiles[s] = hpool.tile([128, B1_ROWS, W1], mm_dt, tag="h",
                                    name=f"H{s}")
        H = H_tiles[s]
        B1v = B1_tiles[s].rearrange("p (r c) -> p r c", c=W0)
        P1s = []
        for i in range(2):
            j = 2 * g + i
            if j >= NC1:
                break
            P1 = ps1.tile([128, 2, W1], f32, tag="p1", name=f"P1_{i}")
            nc.tensor.matmul(
                P1[:], w1sb[32 * i:32 * i + 27, :],
                B1v[32 * i:32 * i + 27, 2 * j:2 * j + 2, 2:W0],
                start=True, stop=True, tile_position=(32 * i, 0))
            P1s.append((j, P1))
        for j, P1 in P1s:
            r = 2 * j
            if j % 2 == 0:
                nc.vector.tensor_copy(H[0:C1, r:r + 2, :], P1[0:C1])
            else:
                nc.scalar.copy(H[0:C1, r:r + 2, :], P1[0:C1])

    # row-shift pieces: H[64:128] rows [a,b) = h rows [a+1,b+1), copied
    # from H's lower half.  Piece p runs after pair-group 2p+1 which has
    # produced h rows < 8p+10.
    H1_PIECES = [(0, 7), (7, 15), (15, 23), (23, 31), (31, B1_ROWS - 1)]

    def emit_h1_piece(s, p):
        a, b = H1_PIECES[p]
        H = H_tiles[s]
        for h0, h1, eng in ((0, 32, nc.sync), (32, C1, nc.scalar)):
            eng.dma_start(H[C1 + h0:C1 + h1, a:b, :],
                          H[h0:h1, a + 1:b + 1, :])

    def emit_conv2_block(s, k):
        n, y0 = strip_of(s)
        H = H_tiles[s]
        OS = opool.tile([C2, 2 * BLK, W2], mm_dt, tag="os")
        for c in range(BLK):
            cc = BLK * k + c
            t = 2 * cc
            P2 = ps2.tile([C2, 2, W2], f32, tag="p2", name=f"P2_{c}")
            for dj in range(3):
                nc.tensor.matmul(P2[:], wp_sb[dj][:],
                                 H[:, t:t + 2, dj:dj + W2],
                                 start=(dj == 0), stop=False,
                                 skip_group_check=True)
            for dj in range(3):
                nc.tensor.matmul(P2[:], ws_sb[dj][:],
                                 H[:, t + 1:t + 3, dj:dj + W2],
                                 start=False, stop=(dj == 2),
                                 skip_group_check=True)
            if cc % 2 == 0:
                nc.vector.tensor_copy(OS[:, 2 * c:2 * c + 2, :], P2[:])
            else:
                nc.scalar.copy(OS[:, 2 * c:2 * c + 2, :], P2[:])
        y = y0 + 2 * BLK * k
        # 4 chains so 4 DMA engines carry the output in parallel
        for h0, h1, eng in ((0, 32, nc.sync), (32, 64, nc.scalar),
                            (64, 96, nc.sync), (96, 128, nc.scalar)):
            eng.dma_start(out[n, h0:h1, y:y + 2 * BLK, :], OS[h0:h1])

    # prologue: strips 0/1 im2col, all of conv1(0).
    emit_im2col(0)
    emit_im2col(1)
    for g in range(NG):
        emit_conv1_pair(0, g)
        if g % 2 == 1:
            emit_h1_piece(0, g // 2)

    for s in range(NSTRIPS):
        if s + 2 < NSTRIPS:
            emit_im2col(s + 2)
        for k in range(NB):
            emit_conv2_block(s, k)
            if k < 5 and s + 1 < NSTRIPS:
                emit_conv1_pair(s + 1, 2 * k)
                emit_conv1_pair(s + 1, 2 * k + 1)
                emit_h1_piece(s + 1, k)


def build(mm_dt=None):
    if mm_dt is None:
        mm_dt = _mm_dt()
    nc = bacc.Bacc("TRN2", target_bir_lowering=False, debug=False,
                   num_devices=N_CORES)
    x = nc.dram_tensor("x", [NPC, C0, H0, W0], mm_dt,
                       kind="ExternalInput").ap()
    w1q = nc.dram_tensor("w1q", [27, 128], mm_dt, kind="ExternalInput").ap()
    wp = nc.dram_tensor("wp", [3, 128, C2], mm_dt, kind="ExternalInput").ap()
    ws = nc.dram_tensor("ws", [3, 128, C2], mm_dt, kind="ExternalInput").ap()
    out = nc.dram_tensor("out", [NPC, C2, H2, W2], mm_dt,
                         kind="ExternalOutput").ap()
    with tile.TileContext(nc) as tc:
        with ExitStack() as ctx:
            _emit(ctx, tc, out, x, w1q, wp, ws, mm_dt)
    nc.compile()
    return nc


def host_round(a: np.ndarray) -> np.ndarray:
    """Cast fp32 to the matmul storage dtype (bf16 cast, or tf32 rounding)."""
    a = np.ascontiguousarray(a, dtype=np.float32)
    if MODE == "bf16":
        return a.astype(ml_dtypes.bfloat16)
    b = a.view(np.uint32).copy()
    b += 0xFFF + ((b >> 13) & 1)
    b &= np.uint32(0xFFFFE000)
    return b.view(np.float32)


def pack_weights(w1: np.ndarray, w2: np.ndarray):
    """Host-side repack so every device DMA is contiguous.

    w1q[p, o] = w1[o%64, c, di, dj], p = (di*3+dj)*3 + c  (cols duplicated)
    wp[dj, k, o]: k<64 -> w2[o, k, 0, dj]; k>=64 -> w2[o, k-64, 1, dj]
    ws[dj, k, o]: k<64 -> 0;              k>=64 -> w2[o, k-64, 2, dj]
    """
    w1 = np.ascontiguousarray(np.asarray(w1), dtype=np.float32)
    w2 = np.ascontiguousarray(np.asarray(w2), dtype=np.float32)
    w1t = w1.transpose(2, 3, 1, 0).reshape(27, C1)
    w1q = np.concatenate([w1t, w1t], axis=1)
    wp = np.empty((3, 128, C2), np.float32)
    wp[:, :C1] = w2[:, :, 0, :].transpose(2, 1, 0)
    wp[:, C1:] = w2[:, :, 1, :].transpose(2, 1, 0)
    ws = np.zeros((3, 128, C2), np.float32)
    ws[:, C1:] = w2[:, :, 2, :].transpose(2, 1, 0)
    return host_round(w1q), host_round(wp), host_round(ws)


_NC_CACHE: dict = {}


def _get_nc():
    key = ("main", MODE, TY)
    if key not in _NC_CACHE:
        _NC_CACHE[key] = build()
    return _NC_CACHE[key]


def run(x, w1, w2, trace: bool = False):
    """Shard, run on 8 cores, gather.  Returns (out, BassKernelResults)."""
    x = np.ascontiguousarray(np.asarray(x), dtype=np.float32)
    assert x.shape == (FULL_N, C0, H0, W0), x.shape
    w1q, wp, ws = pack_weights(w1, w2)
    xs = host_round(x).reshape(N_CORES, NPC, C0, H0, W0)
    in_maps = [
        {"x": np.ascontiguousarray(xs[c]), "w1q": w1q, "wp": wp, "ws": ws}
        for c in range(N_CORES)
    ]
    nc = _get_nc()
    res = bass_utils.run_bass_kernel_spmd(
        nc, in_maps, core_ids=list(range(N_CORES)), trace=trace)
    out = np.concatenate(
        [np.asarray(r["out"], dtype=np.float32) for r in res.results], axis=0)
    return out, res


def kernel(x, w1, w2):
    out, _ = run(x, w1, w2, trace=False)
    return out
